# revision 1
# baseline (speedup 1.0000x reference)
"""GCNConv (gnn_message_passing) on 8 Trainium2 NeuronCores — v4.

out = D^{-1/2} (A + I) D^{-1/2} (X W) + b

Host folds dinv into x rows (prescale), so the device computes
  h = (dinv*x) @ W          (bf16, replicated on every core)
  out[d] = dinv[d] * sum_{e: dst=d} h[src_e]     (self loops are edges)

Device plan (SPMD, one program per core, no collectives):
  Phase A (replicated): every core computes h for ALL nodes.
    xsT [256, NPAD] bf16 (host: prescaled + transposed + zero-padded).
    W stationary [128cin, 64], stream xsT N=512 -> hT [64, n] in PSUM,
    cast bf16, xbar dma-transpose to node-major, store h_all [NPAD, 64]
    bf16 in local DRAM.
  Phase B: edges partitioned by dst shard (12500 dst/core), 128-edge
    slot groups per 128-dst window (group count = max over cores, so one
    static program serves all 8).  Per group: one indirect DMA gather
    (128 row descriptors, the only offset shape the base SWDGE vector-
    DMA ucode supports), round-robined over num_swdge_queues SWDGE
    queues for parallel descriptor generation.  Per window: one 3D
    bf16 is_equal builds all one-hot segs at once; per group a matmul
    ps[128dst, 64] += seg_g^T @ msg_g accumulates; scale by dinv[dst],
    store node-major bf16.  Pad slots gather row N (zero pad region of
    xsT => h=0) and have doff=-1 so the one-hot kills them.
    Host: upcast + bias.
"""

import numpy as np

P = 128
COUT = 64
CH = 4096          # phase A node chunk
NQ = 4             # SWDGE queues for gather generation


def _cdiv(a, b):
    return -(-a // b)


# ----------------------------------------------------------------------------
# CPU planning
# ----------------------------------------------------------------------------
def _plan(edge_index, N, ncores):
    shard = N // ncores
    nwin = _cdiv(shard, P)
    npad = _cdiv(N, CH) * CH if N % CH else N + CH

    src = np.asarray(edge_index[0], dtype=np.int64)
    dst = np.asarray(edge_index[1], dtype=np.int64)
    deg = np.bincount(dst, minlength=N).astype(np.float64) + 1.0
    dinv = (1.0 / np.sqrt(deg)).astype(np.float32)

    loop = np.arange(N, dtype=np.int64)
    src = np.concatenate([src, loop])
    dst = np.concatenate([dst, loop])

    per_core = []
    counts = np.zeros((ncores, nwin), np.int64)
    for c in range(ncores):
        m = (dst // shard) == c
        s = src[m]
        d = dst[m] - c * shard
        w = d >> 7
        order = np.argsort(w, kind="stable")
        s, d, w = s[order], d[order], w[order]
        np.add.at(counts[c], w, 1)
        per_core.append((s, d, w))

    G = _cdiv(counts.max(axis=0), P)        # groups per window [nwin]
    gbase = np.concatenate([[0], np.cumsum(G)])
    GT = int(G.sum())
    slots = GT * P

    goff = np.empty((ncores, P, GT), np.int32)
    dstoff = np.empty((ncores, P, GT), np.float32)
    for c in range(ncores):
        s, d, w = per_core[c]
        runstart = np.concatenate([[0], 1 + np.flatnonzero(w[1:] != w[:-1])])
        rank = np.arange(len(w)) - np.repeat(
            runstart, np.diff(np.concatenate([runstart, [len(w)]])))
        slot = (gbase[w] + (rank >> 7)) * P + (rank & 127)
        g = np.full(slots, N, np.int64)         # pad -> zero row at N
        g[slot] = s
        doff = np.full(slots, -1.0, np.float32)
        doff[slot] = (d - (w << 7)).astype(np.float32)
        goff[c] = g.astype(np.int32).reshape(GT, P).T
        dstoff[c] = doff.reshape(GT, P).T

    return dict(shard=shard, nwin=nwin, npad=npad, G=G, gbase=gbase, GT=GT,
                dinv=dinv, goff=goff, dstoff=dstoff)


# ----------------------------------------------------------------------------
# Device program
# ----------------------------------------------------------------------------
def _build(plan, N, CIN, ncores, unroll=1, skip=()):
    import concourse.bacc as bacc
    import concourse.tile as tile
    import concourse.bass as bass
    import concourse.mybir as mybir

    f32 = mybir.dt.float32
    bf16 = mybir.dt.bfloat16
    shard, nwin, npad = plan["shard"], plan["nwin"], plan["npad"]
    G, gbase, GT = plan["G"], plan["gbase"], plan["GT"]
    padn = nwin * P
    kblk = CIN // P
    GMAX = int(G.max())

    nc = bacc.Bacc("TRN2", target_bir_lowering=False, debug=False,
                   enable_asserts=False, num_devices=ncores,
                   num_swdge_queues=NQ)

    xsT_in = nc.dram_tensor("xsT", [CIN, npad], bf16, kind="ExternalInput")
    w_in = nc.dram_tensor("w2", [P, kblk * COUT], bf16, kind="ExternalInput")
    goff_in = nc.dram_tensor("goff", [P, GT], mybir.dt.int32,
                             kind="ExternalInput")
    doff_in = nc.dram_tensor("dstoff", [P, GT], bf16, kind="ExternalInput")
    iota_in = nc.dram_tensor("iota", [P, P], bf16, kind="ExternalInput")
    dinv_in = nc.dram_tensor("dinv_t", [P, nwin], f32, kind="ExternalInput")
    out_t = nc.dram_tensor("out", [padn, COUT], bf16, kind="ExternalOutput")

    qi = 0

    with tile.TileContext(nc) as tc:
        with (
            tc.tile_pool(name="dram", bufs=1, space="DRAM") as dram,
            tc.tile_pool(name="const", bufs=1) as const,
            tc.tile_pool(name="xp", bufs=2) as xp,
            tc.tile_pool(name="hT", bufs=2) as hTp,
            tc.tile_pool(name="hnm", bufs=2) as hnmp,
            tc.tile_pool(name="msg", bufs=3) as msgp,
            tc.tile_pool(name="seg", bufs=2) as segp,
            tc.tile_pool(name="osb", bufs=4) as osbp,
            tc.tile_pool(name="psA", bufs=2, space="PSUM") as psA,
            tc.tile_pool(name="psB", bufs=4, space="PSUM") as psB,
        ):
            h_all = dram.tile([npad, COUT], bf16)

            for _it in range(unroll):
                w_sb = const.tile([P, kblk * COUT], bf16, tag="w_sb")
                nc.sync.dma_start(w_sb[:], w_in[:, :])
                iota_sb = const.tile([P, P], bf16, tag="iota")
                nc.sync.dma_start(iota_sb[:], iota_in[:, :])
                goff_sb = const.tile([P, GT], mybir.dt.int32, tag="goff")
                nc.sync.dma_start(goff_sb[:], goff_in[:, :])
                doff_sb = const.tile([P, GT], bf16, tag="doff")
                nc.sync.dma_start(doff_sb[:], doff_in[:, :])
                dinv_sb = const.tile([P, nwin], f32, tag="dinv")
                nc.sync.dma_start(dinv_sb[:], dinv_in[:, :])

                # ---------------- Phase A: h = xs @ W (replicated) --------
                c0 = 0
                while c0 < npad:
                    ch = min(CH, npad - c0)
                    xt = xp.tile([P, kblk, CH], bf16, tag="xt")
                    nc.sync.dma_start(
                        xt[:, :, :ch],
                        xsT_in[:, c0:c0 + ch].rearrange(
                            "(k p) n -> p k n", p=P))
                    hT_sb = hTp.tile([COUT, CH], bf16, tag="hT")
                    for s in range(ch // 512):
                        hT_ps = psA.tile([COUT, 512], f32)
                        for k in range(kblk):
                            nc.tensor.matmul(
                                out=hT_ps[:],
                                lhsT=w_sb[:, k * COUT:(k + 1) * COUT],
                                rhs=xt[:, k, s * 512:(s + 1) * 512],
                                start=(k == 0), stop=(k == kblk - 1))
                        nc.vector.tensor_copy(
                            out=hT_sb[:, s * 512:(s + 1) * 512],
                            in_=hT_ps[:])
                    hnm = hnmp.tile([P, CH // P, COUT], bf16, tag="hnm")
                    nc.sync.dma_start_transpose(
                        hnm[:, :ch // P, :], hT_sb[:, :ch])
                    nc.sync.dma_start(
                        h_all[c0:c0 + ch, :].rearrange(
                            "(b p) q -> p b q", p=P),
                        hnm[:, :ch // P, :])
                    c0 += ch

                # ---------------- Phase B: gather + one-hot matmul --------
                for w in (range(nwin) if "phaseB" not in skip else []):
                    gw = int(G[w])
                    gb = int(gbase[w])
                    msg = msgp.tile([P, GMAX, COUT], bf16, tag="msg")
                    for g in range(gw):
                        inst = nc.gpsimd.indirect_dma_start(
                            out=msg[:, g, :], out_offset=None,
                            in_=h_all[:, :],
                            in_offset=bass.IndirectOffsetOnAxis(
                                ap=goff_sb[:, gb + g:gb + g + 1], axis=0))
                        if qi % NQ:
                            inst.ins.queue = f"qPoolDynamic{qi % NQ}"
                        qi += 1
                    seg = segp.tile([P, GMAX, P], bf16, tag="seg")
                    nc.vector.tensor_tensor(
                        out=seg[:, :gw, :],
                        in0=doff_sb[:, gb:gb + gw, None]
                            .to_broadcast([P, gw, P]),
                        in1=iota_sb[:, None, :].to_broadcast([P, gw, P]),
                        op=mybir.AluOpType.is_equal)
                    ps = psB.tile([P, COUT], f32)
                    for g in range(gw):
                        nc.tensor.matmul(
                            out=ps[:], lhsT=seg[:, g, :],
                            rhs=msg[:, g, :],
                            start=(g == 0), stop=(g == gw - 1))
                    o_sb = osbp.tile([P, COUT], bf16, tag="osb")
                    nc.vector.tensor_scalar_mul(o_sb[:], ps[:],
                                                dinv_sb[:, w:w + 1])
                    nc.sync.dma_start(out_t[w * P:(w + 1) * P, :], o_sb[:])

    nc.compile()
    return nc


# ----------------------------------------------------------------------------
# Entry point
# ----------------------------------------------------------------------------
def _prepare(x, edge_index, W, b, ncores=8):
    from concourse import mybir

    bf16 = mybir.dt.np(mybir.dt.bfloat16)
    x = np.asarray(x)
    W = np.asarray(W)
    N, CIN = x.shape
    plan = _plan(edge_index, N, ncores)
    shard, nwin, npad = plan["shard"], plan["nwin"], plan["npad"]
    dinv = plan["dinv"]

    xsT = np.zeros((CIN, npad), dtype=bf16)
    xsT[:, :N] = (x.astype(np.float32) * dinv[:, None]).T.astype(bf16)
    w2 = np.concatenate([W[:P, :], W[P:, :]], axis=1).astype(bf16)
    iota = np.tile(np.arange(P, dtype=np.float32), (P, 1)).astype(bf16)

    in_maps = []
    for c in range(ncores):
        dv = np.zeros((nwin * P,), np.float32)
        dv[:shard] = dinv[c * shard:(c + 1) * shard]
        in_maps.append({
            "xsT": xsT,
            "w2": w2,
            "goff": np.ascontiguousarray(plan["goff"][c]),
            "dstoff": plan["dstoff"][c].astype(bf16),
            "iota": iota,
            "dinv_t": np.ascontiguousarray(dv.reshape(nwin, P).T),
        })
    return plan, in_maps


def kernel(x, edge_index, W, b, _trace=False):
    from concourse.bass_utils import run_bass_kernel_spmd

    x = np.asarray(x)
    W = np.asarray(W)
    b = np.asarray(b)
    N, CIN = x.shape
    ncores = 8
    plan, in_maps = _prepare(x, edge_index, W, b, ncores)
    shard = plan["shard"]

    nc = _build(plan, N, CIN, ncores)

    kernel.last_build = lambda unroll: (
        nc if unroll == 1 else _build(plan, N, CIN, ncores, unroll=unroll))
    kernel.last_in_maps = in_maps
    res = run_bass_kernel_spmd(nc, in_maps, core_ids=list(range(ncores)))
    out = np.concatenate(
        [r["out"][:shard].astype(np.float32) for r in res.results], axis=0)
    out = out + b.astype(np.float32)
    kernel.last_results = res
    return out



# revision 4
# speedup vs baseline: 2.0138x; 2.0138x over previous
"""GCNConv (gnn_message_passing) on 8 Trainium2 NeuronCores — v5.

out = D^{-1/2} (A + I) D^{-1/2} (X W) + b

Host folds dinv into x rows (prescale), so the device computes
  h = (dinv*x) @ W          (f32 table, replicated on every core)
  out[d] = dinv[d] * sum_{e: dst=d} h[src_e]     (self loops are edges)

v5 changes vs v4: the per-128-edge indirect_dma_start gather (3223 SWDGE
ops/core x ~1us fixed Q7 cost = 4ms serial GPSIMD) is replaced by batched
gpsimd.dma_gather (~100 ops/core).  dma_gather constraints drive the rest:
  * elem_size_bytes % 256 == 0  -> h table is f32 [npad, 64] (256B rows).
    Matmul rhs uses a stride-2 bf16 bitcast view of the gathered f32 tile
    (bf16 == high half of f32), so no cast pass is needed.
  * int16 indices -> the table is split into 4 banks of 32768 rows; edges
    are grouped per (dst-window, bank), each segment padded to a multiple
    of 128 slots (pad slots gather bank row 0 and carry doff=-1 so the
    one-hot kills them).
  * gathers are batched per (superwindow of SW windows, bank): one op
    covers all those windows' slots for that bank (msg tile laid out
    bank-major).  doff (one-hot keys) is laid out window-major so each
    window still needs only one is_equal; a static map translates window
    group j -> msg tile column.
Phase A computes h node-major directly: stationary xsT chunk [128cin,
128nodes], moving W [128cin, 64] -> PSUM [128 nodes, 64] f32, ACT copy to
SBUF, HWDGE store (no dma-transpose; that path is bf16-only).
Accumulation per window: ps[128dst, 64] += seg_g^T @ msg_g (bf16), scale
by dinv[dst], store f32 node-major.  Host: concat + bias.
"""

import numpy as np

P = 128
COUT = 64
CH = 4096          # phase A node chunk
NQ = 4             # SWDGE queues
BANK = 32768       # dma_gather int16 index reach (rows per bank)
SW = 4             # windows per superwindow (gather batch)


def _cdiv(a, b):
    return -(-a // b)


# ----------------------------------------------------------------------------
# CPU planning
# ----------------------------------------------------------------------------
def _plan(edge_index, N, ncores):
    shard = N // ncores
    nwin = _cdiv(shard, P)
    npad = _cdiv(N, CH) * CH if N % CH else N + CH   # h table rows
    nbank = _cdiv(npad, BANK)
    nsw = _cdiv(nwin, SW)

    src = np.asarray(edge_index[0], dtype=np.int64)
    dst = np.asarray(edge_index[1], dtype=np.int64)
    deg = np.bincount(dst, minlength=N).astype(np.float64) + 1.0
    dinv = (1.0 / np.sqrt(deg)).astype(np.float32)

    loop = np.arange(N, dtype=np.int64)
    src = np.concatenate([src, loop])
    dst = np.concatenate([dst, loop])

    # per-core edge lists sorted by (window, bank)
    per_core = []
    cnt = np.zeros((ncores, nwin, nbank), np.int64)
    for c in range(ncores):
        m = (dst // shard) == c
        s = src[m]
        d = dst[m] - c * shard
        w = d >> 7
        b = s // BANK
        order = np.lexsort((b, w))
        s, d, w, b = s[order], d[order], w[order], b[order]
        np.add.at(cnt[c], (w, b), 1)
        per_core.append((s, d, w, b))

    G = _cdiv(cnt.max(axis=0), P)            # [nwin, nbank] groups (uniform)

    # global group order: (sw, bank, window)  -> msg tile layout
    # doff order:         (sw, window, bank)  -> one is_equal per window
    gslot = np.zeros((nwin, nbank), np.int64)    # global group base (msg order)
    gdoff = np.zeros((nwin, nbank), np.int64)    # global doff column base
    swinfo = []     # per sw: dict(slot0, nslots, banks=[(b, lg0, lg1)])
    win_midx = []   # per window: list of msg-order group indices (doff order)
    win_d0 = []     # per window: doff col base
    gpos = 0
    for si in range(nsw):
        ws = range(si * SW, min((si + 1) * SW, nwin))
        sw_g0 = gpos
        banks = []
        for b in range(nbank):
            lg0 = gpos - sw_g0
            for w in ws:
                gslot[w, b] = gpos
                gpos += G[w, b]
            lg1 = gpos - sw_g0
            if lg1 > lg0:
                banks.append((b, lg0, lg1))
        swinfo.append(dict(g0=sw_g0, ng=gpos - sw_g0, banks=banks))
    dpos = 0
    for w in range(nwin):
        win_d0.append(dpos)
        mids = []
        for b in range(nbank):
            mids.extend(range(int(gslot[w, b]), int(gslot[w, b] + G[w, b])))
        win_midx.append(mids)
        dpos += len(mids)
    GT = gpos
    assert dpos == GT
    slots = GT * P

    # slot-level arrays
    idx16 = np.zeros((ncores, 128, slots // 16), np.int16)
    dstoff = np.empty((ncores, P, GT), np.float32)
    for c in range(ncores):
        s, d, w, b = per_core[c]
        gidx = np.full(slots, 0, np.int64)       # bank-relative row (pad -> 0)
        doff = np.full(slots, -1.0, np.float32)  # dst-in-window (pad -> -1)
        # rank within each (w, b) run
        wb = w * nbank + b
        runstart = np.concatenate([[0], 1 + np.flatnonzero(wb[1:] != wb[:-1])])
        rank = np.arange(len(wb)) - np.repeat(
            runstart, np.diff(np.concatenate([runstart, [len(wb)]])))
        slot = (gslot[w, b] + (rank >> 7)) * P + (rank & 127)
        gidx[slot] = s - b * BANK
        # doff column for edge: win_d0[w] + (bank-local group order)
        # bank-local group base within window w's doff block:
        dbase_wb = np.zeros((nwin, nbank), np.int64)
        for w_ in range(nwin):
            acc = win_d0[w_]
            for b_ in range(nbank):
                dbase_wb[w_, b_] = acc
                acc += G[w_, b_]
        dslot = (dbase_wb[w, b] + (rank >> 7)) * P + (rank & 127)
        dofftmp = np.full(slots, -1.0, np.float32)
        dofftmp[dslot] = (d & 127).astype(np.float32)
        dstoff[c] = dofftmp.reshape(GT, P).T
        # idx16: slot j -> [p%16 == j%16, col j//16], replicated across 8 groups
        a = gidx.astype(np.int16).reshape(slots // 16, 16).T   # [16, slots/16]
        idx16[c] = np.tile(a, (8, 1))

    return dict(shard=shard, nwin=nwin, npad=npad, nbank=nbank, nsw=nsw,
                G=G, GT=GT, slots=slots, swinfo=swinfo, win_midx=win_midx,
                win_d0=win_d0, dinv=dinv, idx16=idx16, dstoff=dstoff)


# ----------------------------------------------------------------------------
# Device program
# ----------------------------------------------------------------------------
def _build(plan, N, CIN, ncores, unroll=1, skip=()):
    import concourse.bacc as bacc
    import concourse.tile as tile
    import concourse.bass as bass
    import concourse.mybir as mybir

    f32 = mybir.dt.float32
    bf16 = mybir.dt.bfloat16
    i16 = mybir.dt.int16
    nwin, npad, nbank = plan["nwin"], plan["npad"], plan["nbank"]
    G, GT, slots = plan["G"], plan["GT"], plan["slots"]
    swinfo, win_midx, win_d0 = plan["swinfo"], plan["win_midx"], plan["win_d0"]
    padn = nwin * P
    kblk = CIN // P
    swGmax = max(sw["ng"] for sw in swinfo)
    wGmax = max(len(m) for m in win_midx)

    nc = bacc.Bacc("TRN2", target_bir_lowering=False, debug=False,
                   enable_asserts=False, num_devices=ncores,
                   num_swdge_queues=NQ)

    xsT_in = nc.dram_tensor("xsT", [CIN, npad], bf16, kind="ExternalInput")
    w_in = nc.dram_tensor("w2", [P, kblk * COUT], bf16, kind="ExternalInput")
    idx_in = nc.dram_tensor("gidx", [P, slots // 16], i16,
                            kind="ExternalInput")
    doff_in = nc.dram_tensor("dstoff", [P, GT], bf16, kind="ExternalInput")
    iota_in = nc.dram_tensor("iota", [P, P], bf16, kind="ExternalInput")
    dinv_in = nc.dram_tensor("dinv_t", [P, nwin], f32, kind="ExternalInput")
    out_t = nc.dram_tensor("out", [padn, COUT], f32, kind="ExternalOutput")

    qi = 0

    with tile.TileContext(nc) as tc:
        with (
            tc.tile_pool(name="dram", bufs=1, space="DRAM") as dram,
            tc.tile_pool(name="const", bufs=1) as const,
            tc.tile_pool(name="xp", bufs=2) as xp,
            tc.tile_pool(name="hsb", bufs=3) as hsp,
            tc.tile_pool(name="idx", bufs=2) as idxp,
            tc.tile_pool(name="msgf", bufs=2) as msgfp,
            tc.tile_pool(name="seg", bufs=3) as segp,
            tc.tile_pool(name="osb", bufs=4) as osbp,
            tc.tile_pool(name="psA", bufs=2, space="PSUM") as psA,
            tc.tile_pool(name="psB", bufs=4, space="PSUM") as psB,
        ):
            h2 = dram.tile([npad, COUT], f32)

            for _it in range(unroll):
                w_sb = const.tile([P, kblk * COUT], bf16, tag="w_sb")
                nc.sync.dma_start(w_sb[:], w_in[:, :])
                iota_sb = const.tile([P, P], bf16, tag="iota")
                nc.sync.dma_start(iota_sb[:], iota_in[:, :])
                doff_sb = const.tile([P, GT], bf16, tag="doff")
                nc.sync.dma_start(doff_sb[:], doff_in[:, :])
                dinv_sb = const.tile([P, nwin], f32, tag="dinv")
                nc.sync.dma_start(dinv_sb[:], dinv_in[:, :])

                # ---------------- Phase A: h = xs @ W (replicated, f32) ----
                for c0 in range(0, npad, CH):
                    xt = xp.tile([P, kblk, CH], bf16, tag="xt")
                    nc.sync.dma_start(
                        xt[:],
                        xsT_in[:, c0:c0 + CH].rearrange(
                            "(k p) n -> p k n", p=P))
                    for s in range(CH // 1024):
                        ps = psA.tile([P, 8, COUT], f32)
                        for t in range(8):
                            base = s * 1024 + t * P
                            for k in range(kblk):
                                nc.tensor.matmul(
                                    out=ps[:, t, :],
                                    lhsT=xt[:, k, base:base + P],
                                    rhs=w_sb[:, k * COUT:(k + 1) * COUT],
                                    start=(k == 0), stop=(k == kblk - 1))
                        hsb = hsp.tile([P, 8, COUT], f32, tag="hsb")
                        nc.scalar.copy(hsb[:], ps[:])
                        nc.sync.dma_start(
                            h2[c0 + s * 1024:c0 + (s + 1) * 1024, :]
                            .rearrange("(b p) q -> p b q", p=P),
                            hsb[:])

                # ---------------- Phase B: batched gather + one-hot matmul -
                if "phaseB" in skip:
                    continue
                slot0 = 0
                for si, sw in enumerate(swinfo):
                    ns = sw["ng"] * P
                    idx_sb = idxp.tile([P, swGmax * 8], i16, tag="idx")
                    nc.sync.dma_start(
                        idx_sb[:, :ns // 16],
                        idx_in[:, slot0 // 16:(slot0 + ns) // 16])
                    msgf = msgfp.tile([P, swGmax, COUT], f32, tag="msgf")
                    for (b, lg0, lg1) in sw["banks"]:
                        b0 = b * BANK
                        b1 = min(b0 + BANK, npad)
                        inst = nc.gpsimd.dma_gather(
                            out_ap=msgf[:, lg0:lg1, :],
                            in_ap=h2[b0:b1, :],
                            idxs_ap=idx_sb[:, lg0 * 8:lg1 * 8],
                            num_idxs=(lg1 - lg0) * P,
                            num_idxs_reg=(lg1 - lg0) * P,
                            elem_size=COUT,
                            queue_num=qi % NQ,
                            single_packet=False,
                        )
                        qi += 1
                    ws = range(si * SW, min((si + 1) * SW, nwin))
                    for w in ws:
                        mids = win_midx[w]
                        gw = len(mids)
                        d0 = win_d0[w]
                        seg = segp.tile([P, wGmax, P], bf16, tag="seg")
                        nc.vector.tensor_tensor(
                            out=seg[:, :gw, :],
                            in0=doff_sb[:, d0:d0 + gw, None]
                                .to_broadcast([P, gw, P]),
                            in1=iota_sb[:, None, :].to_broadcast([P, gw, P]),
                            op=mybir.AluOpType.is_equal)
                        ps = psB.tile([P, COUT], f32)
                        for j, m in enumerate(mids):
                            rhs = msgf[:, m - sw["g0"], :].bitcast(bf16)[:, 1::2]
                            nc.tensor.matmul(
                                out=ps[:], lhsT=seg[:, j, :], rhs=rhs,
                                start=(j == 0), stop=(j == gw - 1))
                        o_sb = osbp.tile([P, COUT], f32, tag="osb")
                        nc.vector.tensor_scalar_mul(o_sb[:], ps[:],
                                                    dinv_sb[:, w:w + 1])
                        nc.sync.dma_start(out_t[w * P:(w + 1) * P, :],
                                          o_sb[:])
                    slot0 += ns

    nc.compile()
    return nc


# ----------------------------------------------------------------------------
# Entry point
# ----------------------------------------------------------------------------
def _prepare(x, edge_index, W, b, ncores=8):
    from concourse import mybir

    bf16 = mybir.dt.np(mybir.dt.bfloat16)
    x = np.asarray(x)
    W = np.asarray(W)
    N, CIN = x.shape
    plan = _plan(edge_index, N, ncores)
    shard, nwin, npad = plan["shard"], plan["nwin"], plan["npad"]
    dinv = plan["dinv"]

    xsT = np.zeros((CIN, npad), dtype=bf16)
    xsT[:, :N] = (x.astype(np.float32) * dinv[:, None]).T.astype(bf16)
    w2 = np.concatenate([W[:P, :], W[P:, :]], axis=1).astype(bf16)
    iota = np.tile(np.arange(P, dtype=np.float32), (P, 1)).astype(bf16)

    in_maps = []
    for c in range(ncores):
        dv = np.zeros((nwin * P,), np.float32)
        dv[:shard] = dinv[c * shard:(c + 1) * shard]
        in_maps.append({
            "xsT": xsT,
            "w2": w2,
            "gidx": np.ascontiguousarray(plan["idx16"][c]),
            "dstoff": plan["dstoff"][c].astype(bf16),
            "iota": iota,
            "dinv_t": np.ascontiguousarray(dv.reshape(nwin, P).T),
        })
    return plan, in_maps


def kernel(x, edge_index, W, b, _trace=False):
    from concourse.bass_utils import run_bass_kernel_spmd

    x = np.asarray(x)
    W = np.asarray(W)
    b = np.asarray(b)
    N, CIN = x.shape
    ncores = 8
    plan, in_maps = _prepare(x, edge_index, W, b, ncores)
    shard = plan["shard"]

    nc = _build(plan, N, CIN, ncores)

    kernel.last_build = lambda unroll: (
        nc if unroll == 1 else _build(plan, N, CIN, ncores, unroll=unroll))
    kernel.last_in_maps = in_maps
    res = run_bass_kernel_spmd(nc, in_maps, core_ids=list(range(ncores)))
    out = np.concatenate(
        [r["out"][:shard].astype(np.float32) for r in res.results], axis=0)
    out = out + b.astype(np.float32)
    kernel.last_results = res
    return out
